# revision 1
# baseline (speedup 1.0000x reference)
"""KNN cluster kernel for Trainium2 (8 NeuronCores, one batch per core).

Computes, for each of N=8 batches independently: squared L2 distances between
queries coords2[:, n, :] (L2=4096) and references coords1[:, n, :] (L1=4096)
in C=64 dims, then the indices of the 16 nearest references per query
(ascending distance). Output matches torch_cluster.knn-style flattened
(clusters, batch_idx) of the jax reference.

Device strategy per core:
  - Load Q (4096,64) and X (4096,64); row norms q2/x2 on the scalar engine.
  - Augmented transposed operands (K=66): rows 0..63 = Q^T / 2*X^T plus rows
    encoding (-q2)/(-x2), so one matmul yields s = 2*Q.X - q2 - x2 = -dist^2
  - Top-16 per query row on the DVE, two passes over the 4096-wide row:
    per-chunk max8 + per-chunk find-index8 (16 chunks of 256, chunk-local
    uint16 indices), then a tiny merge over the 128 candidates yields the 16
    winner values and their *positions* in the candidate array.
  - Host maps positions -> global indices (16-of-128 gather per query, plus
    chunk offset), avoiding the two full-width find-index8 scans that would
    otherwise dominate the DVE.
"""

import sys

import numpy as np

sys.path.insert(0, "/opt/trn_rl_repo")

L = 4096  # L1 == L2
N = 8
C = 64
K = 16
P = 128  # partitions / queries per tile
NT = L // P  # 32 query tiles
XC = 8  # matmul moving chunks of 512
MM_N = L // XC  # 512
NCH = 8  # top-k chunking of the 4096-wide row
CHW = L // NCH  # 256
NCAND = NCH * 8  # 128 candidates per query
KAUG = C + 2  # 66: contraction with -q2 / -x2 rows folded in
NEG_INF = -1.0e30

_CACHE = {}


def build_body(tc, q_ap, x_ap, ci_ap, pos_ap):
    from concourse import mybir, masks

    nc = tc.nc
    f32 = mybir.dt.float32
    f32r = mybir.dt.float32r
    u16 = mybir.dt.uint16

    with (
        tc.tile_pool(name="const", bufs=1) as const_pool,
        tc.tile_pool(name="inp", bufs=1) as inp_pool,
        tc.tile_pool(name="aug", bufs=1) as aug_pool,
        tc.tile_pool(name="tpsum", bufs=2, space="PSUM") as tpsum_pool,
        tc.tile_pool(name="mpsum", bufs=4, space="PSUM") as mpsum_pool,
        tc.tile_pool(name="s", bufs=3) as s_pool,
        tc.tile_pool(name="small", bufs=2) as small_pool,
    ):
        ident = const_pool.tile([P, P], f32)
        masks.make_identity(nc, ident[:])

        q_sb = inp_pool.tile([P, NT * C], f32)
        x_sb = inp_pool.tile([P, NT * C], f32)
        sqd = inp_pool.tile([P, C], f32)
        q2 = inp_pool.tile([P, NT], f32)
        x2 = inp_pool.tile([P, NT], f32)

        nc.sync.dma_start(
            q_sb[:].rearrange("p (t c) -> p t c", c=C),
            q_ap.rearrange("(t p) c -> p t c", p=P),
        )
        nc.sync.dma_start(
            x_sb[:].rearrange("p (t c) -> p t c", c=C),
            x_ap.rearrange("(t p) c -> p t c", p=P),
        )

        q3 = q_sb[:].rearrange("p (t c) -> p t c", c=C)
        x3 = x_sb[:].rearrange("p (t c) -> p t c", c=C)

        # q2[p, t] = sum_c Q[t*128+p, c]^2 (scalar engine: square + accum)
        for t in range(NT):
            nc.scalar.activation(
                sqd[:],
                q_sb[:, t * C : (t + 1) * C],
                mybir.ActivationFunctionType.Square,
                accum_out=q2[:, t : t + 1],
            )
        for t in range(NT):
            nc.scalar.activation(
                sqd[:],
                x_sb[:, t * C : (t + 1) * C],
                mybir.ActivationFunctionType.Square,
                accum_out=x2[:, t : t + 1],
            )

        # Augmented pre-transpose layouts [P, NT*KAUG] (float32r):
        #   Q rows: [Q | 1 | -q2]      X rows: [2X | -x2 | 1]
        aug_q = aug_pool.tile([P, NT * KAUG], f32)
        aug_x = aug_pool.tile([P, NT * KAUG], f32)
        aq3 = aug_q[:].rearrange("p (t e) -> p t e", e=KAUG)
        ax3 = aug_x[:].rearrange("p (t e) -> p t e", e=KAUG)
        nc.scalar.copy(aq3[:, :, 0:C], q3)
        nc.any.memset(aq3[:, :, C : C + 1], 1.0)
        nc.scalar.mul(aq3[:, :, C + 1 : C + 2], q2[:].rearrange("p (t o) -> p t o", o=1), -1.0)
        nc.scalar.mul(ax3[:, :, 0:C], x3, 2.0)
        nc.scalar.mul(ax3[:, :, C : C + 1], x2[:].rearrange("p (t o) -> p t o", o=1), -1.0)
        nc.any.memset(ax3[:, :, C + 1 : C + 2], 1.0)

        # Transposed operands [KAUG, L] float32r via PE transpose
        qT = aug_pool.tile([KAUG, L], f32)
        xT = aug_pool.tile([KAUG, L], f32)
        for t in range(NT):
            pq = tpsum_pool.tile([KAUG, P], f32, tag="tps")
            nc.tensor.transpose(pq[:], aug_q[:, t * KAUG : (t + 1) * KAUG], ident[:])
            nc.scalar.copy(qT[:, t * P : (t + 1) * P], pq[:])
            px = tpsum_pool.tile([KAUG, P], f32, tag="tps")
            nc.tensor.transpose(px[:], aug_x[:, t * KAUG : (t + 1) * KAUG], ident[:])
            nc.scalar.copy(xT[:, t * P : (t + 1) * P], px[:])

        # Main loop: per 128-query tile, matmul + two-pass chunked top-16
        for t in range(NT):
            s_sb = s_pool.tile([P, L], f32, tag="s")
            for j in range(XC):
                ps = mpsum_pool.tile([P, MM_N], f32, tag="mm")
                nc.tensor.matmul(
                    ps[:],
                    lhsT=qT[:, t * P : (t + 1) * P],
                    rhs=xT[:, j * MM_N : (j + 1) * MM_N],
                    start=True,
                    stop=True,
                )
                nc.scalar.copy(s_sb[:, j * MM_N : (j + 1) * MM_N], ps[:])

            cand_v = small_pool.tile([P, NCAND], f32, tag="cand_v")
            cand2 = small_pool.tile([P, NCAND], f32, tag="cand2")
            ci_t = small_pool.tile([P, NCAND], u16, tag="ci")
            v16 = small_pool.tile([P, 16], f32, tag="v16")
            pos_t = small_pool.tile([P, 16], u16, tag="pos")
            for ch in range(NCH):
                nc.vector.max(
                    cand_v[:, ch * 8 : (ch + 1) * 8],
                    s_sb[:, ch * CHW : (ch + 1) * CHW],
                )
            for ch in range(NCH):
                nc.vector.max_index(
                    ci_t[:, ch * 8 : (ch + 1) * 8],
                    cand_v[:, ch * 8 : (ch + 1) * 8],
                    s_sb[:, ch * CHW : (ch + 1) * CHW],
                )
            nc.vector.max(v16[:, 0:8], cand_v[:])
            nc.vector.match_replace(cand2[:], v16[:, 0:8], cand_v[:], NEG_INF)
            nc.vector.max(v16[:, 8:16], cand2[:])
            nc.vector.max_index(pos_t[:, 0:8], v16[:, 0:8], cand_v[:])
            nc.vector.max_index(pos_t[:, 8:16], v16[:, 8:16], cand2[:])

            nc.sync.dma_start(ci_ap[t * P : (t + 1) * P, :], ci_t[:])
            nc.sync.dma_start(pos_ap[t * P : (t + 1) * P, :], pos_t[:])


def _build_program():
    from concourse import bacc, mybir, tile

    nc = bacc.Bacc(
        "TRN2",
        target_bir_lowering=False,
        debug=False,
        enable_asserts=True,
        num_devices=N,
    )
    q_dram = nc.dram_tensor("q", [L, C], mybir.dt.float32, kind="ExternalInput")
    x_dram = nc.dram_tensor("x", [L, C], mybir.dt.float32, kind="ExternalInput")
    ci_dram = nc.dram_tensor("ci", [L, NCAND], mybir.dt.uint16, kind="ExternalOutput")
    pos_dram = nc.dram_tensor("pos", [L, K], mybir.dt.uint16, kind="ExternalOutput")

    with tile.TileContext(nc) as tc:
        build_body(tc, q_dram.ap(), x_dram.ap(), ci_dram.ap(), pos_dram.ap())

    nc.compile()
    return nc


def _get_nc():
    if "nc" not in _CACHE:
        _CACHE["nc"] = _build_program()
    return _CACHE["nc"]


def _postprocess(ci, pos):
    # ci: (L, 128) uint16 chunk-local indices; pos: (L, 16) uint16 slots
    slot = pos.astype(np.int64)  # (L, 16), values in [0, 128)
    local = np.take_along_axis(ci.astype(np.int64), slot, axis=1)
    return (slot >> 3) * CHW + local  # global candidate index in [0, 4096)


def kernel(coords1, coords2, k):
    from concourse.bass_utils import run_bass_kernel_spmd

    coords1 = np.asarray(coords1)
    coords2 = np.asarray(coords2)
    assert int(k) == K, f"kernel hardcoded for k={K}, got {k}"
    assert coords1.shape == (L, N, C) and coords2.shape == (L, N, C)

    nc = _get_nc()
    in_maps = [
        {
            "q": np.ascontiguousarray(coords2[:, n, :], dtype=np.float32),
            "x": np.ascontiguousarray(coords1[:, n, :], dtype=np.float32),
        }
        for n in range(N)
    ]
    res = run_bass_kernel_spmd(nc, in_maps, core_ids=list(range(N)))
    local = np.stack(
        [_postprocess(r["ci"], r["pos"]) for r in res.results], axis=0
    )  # (N, L, K)
    # global_idx = local + n*L1 ; clusters = global_idx mod L2 == local (L1==L2)
    clusters = np.transpose(local, (2, 1, 0)).astype(np.int32).reshape(-1)
    batch_idx = np.broadcast_to(
        np.arange(N, dtype=np.int32), (K, L, N)
    ).reshape(-1)
    return clusters, batch_idx



# revision 6
# speedup vs baseline: 1818.6058x; 1818.6058x over previous
"""KNN cluster kernel for Trainium2 (8 NeuronCores, one batch per core).

For each of N=8 batches independently: squared L2 distances between queries
coords2[:, n, :] (L2=4096) and references coords1[:, n, :] (L1=4096) in C=64
dims, then indices of the 16 nearest references per query, ascending
distance. Output matches torch_cluster.knn-style flattened (clusters,
batch_idx) of the jax reference.

Device strategy per core:
  - score s = 2*Q.X^T - x2 (the per-query q2 term is constant along each
    row and cannot change per-query ordering, so it is dropped). One
    augmented contraction (KAUG=65: [2Q | 1] x [X | -x2]) produces s
    directly in PSUM.
  - The PE runs float32r (11-bit mantissa, 1 cycle/row = 4x the fp32
    rate). To keep fp32-level accuracy each transposed operand is split
    hi/lo (hi = f32r-rounded, lo = f32r(exact - hi)) and the matmul
    accumulates three f32r passes into PSUM: qh.xh + qh.xl + ql.xh. The
    dropped ql.xl term is O(2^-24) relative.
  - Per 128-query tile: 2x[128,512] matmul triples fill a [128,1024] PSUM
    tile; the scalar engine copies it to SBUF; the DVE finds per-512-chunk
    top-8 values + chunk-local indices (max8 + max_index8, two passes over
    the row), then a 64-wide merge (max8/match_replace/max8 + 2x
    max_index) gives the 16 winner slots.
  - Outputs: ci (4096x64 u16 chunk-local indices) + pos (4096x16 u16
    winner slots). Host maps slot -> chunk*512 + ci[slot].

Host strategy: the shard_map'd jitted executable is built once and cached;
inputs ship as a single concatenated (8*8192, 64) f32 array (one sharded
transfer), outputs are fetched once each. No zero output-donation buffers
are uploaded (the NEFF writes every output element).
"""

import sys

import numpy as np

sys.path.insert(0, "/opt/trn_rl_repo")

L = 4096  # L1 == L2
N = 8
C = 64
K = 16
P = 128  # partitions / queries per tile
NT = L // P  # 32 query tiles
NCH = 8  # chunks per row
CHW = L // NCH  # 512
NCAND = NCH * 8  # 64 candidates per query
KAUG = C + 1  # 65: contraction with the -x2 row folded in
NEG_INF = -1.0e30

_CACHE = {}


def build_body(tc, qx_ap, ci_ap, pos_ap):
    from concourse import mybir, masks

    nc = tc.nc
    f32 = mybir.dt.float32
    f32r = mybir.dt.float32r
    u16 = mybir.dt.uint16

    q_ap = qx_ap[0:L, :]
    x_ap = qx_ap[L : 2 * L, :]

    with (
        tc.tile_pool(name="const", bufs=1) as const_pool,
        tc.tile_pool(name="inp", bufs=1) as inp_pool,
        tc.tile_pool(name="aug", bufs=1) as aug_pool,
        tc.tile_pool(name="small", bufs=2) as small_pool,
        tc.tile_pool(name="tpsum", bufs=2, space="PSUM") as tpsum_pool,
        tc.tile_pool(name="mpsum", bufs=3, space="PSUM") as mpsum_pool,
        tc.tile_pool(name="srow", bufs=2) as srow_pool,
    ):
        ident = const_pool.tile([P, P], f32)
        masks.make_identity(nc, ident[:])

        q_sb = inp_pool.tile([P, NT * C], f32)
        x_sb = inp_pool.tile([P, NT * C], f32)
        sqd = inp_pool.tile([P, C], f32)
        x2 = inp_pool.tile([P, NT], f32)

        nc.sync.dma_start(
            x_sb[:].rearrange("p (t c) -> p t c", c=C),
            x_ap.rearrange("(t p) c -> p t c", p=P),
        )
        nc.sync.dma_start(
            q_sb[:].rearrange("p (t c) -> p t c", c=C),
            q_ap.rearrange("(t p) c -> p t c", p=P),
        )

        q3 = q_sb[:].rearrange("p (t c) -> p t c", c=C)
        x3 = x_sb[:].rearrange("p (t c) -> p t c", c=C)

        # x2[p, t] = sum_c X[t*128+p, c]^2 (scalar engine: square + accum)
        for t in range(NT):
            nc.scalar.activation(
                sqd[:],
                x_sb[:, t * C : (t + 1) * C],
                mybir.ActivationFunctionType.Square,
                accum_out=x2[:, t : t + 1],
            )

        # Augmented pre-transpose layouts [P, NT*KAUG] (f32, full precision):
        #   Q rows: [2Q | 1]      X rows: [X | -x2]
        aug_q = aug_pool.tile([P, NT * KAUG], f32)
        aug_x = aug_pool.tile([P, NT * KAUG], f32)
        aq3 = aug_q[:].rearrange("p (t e) -> p t e", e=KAUG)
        ax3 = aug_x[:].rearrange("p (t e) -> p t e", e=KAUG)
        nc.scalar.mul(aq3[:, :, 0:C], q3, 2.0)
        nc.any.memset(aq3[:, :, C : C + 1], 1.0)
        nc.scalar.copy(ax3[:, :, 0:C], x3)
        nc.scalar.mul(ax3[:, :, C : C + 1], x2[:].rearrange("p (t o) -> p t o", o=1), -1.0)

        # Transposed operands [KAUG, L], hi/lo f32r split:
        #   hi = f32r-rounded copy of the f32 transpose (scalar engine)
        #   lo = f32r(full - hi)                         (gpsimd engine)
        qT_hi = aug_pool.tile([KAUG, L], f32r)
        qT_lo = aug_pool.tile([KAUG, L], f32r)
        xT_hi = aug_pool.tile([KAUG, L], f32r)
        xT_lo = aug_pool.tile([KAUG, L], f32r)

        def transpose_split(t, aug, dst_hi, dst_lo, which):
            pq = tpsum_pool.tile([KAUG, P], f32, tag="tps", name=f"tp_{which}_{t}")
            nc.tensor.transpose(pq[:], aug[:, t * KAUG : (t + 1) * KAUG], ident[:])
            stage = small_pool.tile([KAUG, P], f32, tag="tstage", name=f"st_{which}_{t}")
            nc.scalar.copy(stage[:], pq[:])
            hi = dst_hi[:, t * P : (t + 1) * P]
            nc.scalar.copy(hi, stage[:])
            nc.gpsimd.tensor_sub(
                dst_lo[:, t * P : (t + 1) * P], stage[:], hi.bitcast(f32)
            )

        # x side fully up front (all tiles' matmuls scan the whole xT);
        # q side: tile 0 now, tile t+1 emitted inside the main loop.
        for t in range(NT):
            transpose_split(t, aug_x, xT_hi, xT_lo, "x")
        transpose_split(0, aug_q, qT_hi, qT_lo, "q")

        # Main loop: per 128-query tile, matmul triples + two-pass top-16
        for t in range(NT):
            cand_v = small_pool.tile([P, NCAND], f32, tag="cand_v")
            cand2 = small_pool.tile([P, NCAND], f32, tag="cand2")
            ci_t = small_pool.tile([P, NCAND], u16, tag="ci")
            v16 = small_pool.tile([P, 16], f32, tag="v16")
            pos_t = small_pool.tile([P, 16], u16, tag="pos")
            s_sb = srow_pool.tile([P, L], f32, tag="s")
            qh = qT_hi[:, t * P : (t + 1) * P]
            ql = qT_lo[:, t * P : (t + 1) * P]
            for h in range(NCH // 2):
                ps = mpsum_pool.tile([P, 2 * CHW], f32, tag="mm")
                for jj in range(2):
                    j = 2 * h + jj
                    dst = ps[:, jj * CHW : (jj + 1) * CHW]
                    xh = xT_hi[:, j * CHW : (j + 1) * CHW]
                    xl = xT_lo[:, j * CHW : (j + 1) * CHW]
                    nc.tensor.matmul(dst, lhsT=qh, rhs=xh, start=True, stop=False)
                    nc.tensor.matmul(dst, lhsT=qh, rhs=xl, start=False, stop=False)
                    nc.tensor.matmul(dst, lhsT=ql, rhs=xh, start=False, stop=True)
                nc.scalar.copy(s_sb[:, h * 2 * CHW : (h + 1) * 2 * CHW], ps[:])
                for jj in range(2):
                    j = 2 * h + jj
                    sl = s_sb[:, j * CHW : (j + 1) * CHW]
                    nc.vector.max(cand_v[:, j * 8 : (j + 1) * 8], sl)
                    nc.vector.max_index(
                        ci_t[:, j * 8 : (j + 1) * 8],
                        cand_v[:, j * 8 : (j + 1) * 8],
                        sl,
                    )
            if t + 1 < NT:
                transpose_split(t + 1, aug_q, qT_hi, qT_lo, "q")
            nc.vector.max(v16[:, 0:8], cand_v[:])
            nc.vector.match_replace(cand2[:], v16[:, 0:8], cand_v[:], NEG_INF)
            nc.vector.max(v16[:, 8:16], cand2[:])
            nc.vector.max_index(pos_t[:, 0:8], v16[:, 0:8], cand_v[:])
            nc.vector.max_index(pos_t[:, 8:16], v16[:, 8:16], cand2[:])

            nc.sync.dma_start(ci_ap[t * P : (t + 1) * P, :], ci_t[:])
            nc.sync.dma_start(pos_ap[t * P : (t + 1) * P, :], pos_t[:])


def _build_program(reps: int = 1):
    from concourse import bacc, mybir, tile

    nc = bacc.Bacc(
        "TRN2",
        target_bir_lowering=False,
        debug=False,
        enable_asserts=True,
        num_devices=N,
    )
    qx_dram = nc.dram_tensor("qx", [2 * L, C], mybir.dt.float32, kind="ExternalInput")
    ci_dram = nc.dram_tensor("ci", [L, NCAND], mybir.dt.uint16, kind="ExternalOutput")
    pos_dram = nc.dram_tensor("pos", [L, K], mybir.dt.uint16, kind="ExternalOutput")

    with tile.TileContext(nc) as tc:
        if reps == 1:
            build_body(tc, qx_dram.ap(), ci_dram.ap(), pos_dram.ap())
        else:
            with tc.For_i(0, reps, 1):
                build_body(tc, qx_dram.ap(), ci_dram.ap(), pos_dram.ap())

    nc.compile()
    return nc


def _get_nc(reps: int = 1):
    key = f"nc{reps}"
    if key not in _CACHE:
        _CACHE[key] = _build_program(reps)
    return _CACHE[key]


def _get_exec(reps: int = 1):
    """Build (once) the jitted shard_map'd executable for the program."""
    key = f"exec{reps}"
    if key in _CACHE:
        return _CACHE[key]

    import jax
    from jax.sharding import Mesh, PartitionSpec
    from jax.experimental.shard_map import shard_map
    from concourse import bass2jax

    nc = _get_nc(reps)
    bass2jax.install_neuronx_cc_hook()
    in_names = ["qx"]
    if nc.partition_id_tensor is not None:
        in_names.append(nc.partition_id_tensor.name)
    in_names = tuple(in_names)

    out_avals = (
        jax.core.ShapedArray((L, NCAND), np.uint16),
        jax.core.ShapedArray((L, K), np.uint16),
    )

    def _body(qx):
        operands = [qx]
        if nc.partition_id_tensor is not None:
            operands.append(bass2jax.partition_id_tensor())
        outs = bass2jax._bass_exec_p.bind(
            *operands,
            out_avals=out_avals,
            in_names=in_names,
            out_names=("ci", "pos"),
            lowering_input_output_aliases=(),
            sim_require_finite=True,
            sim_require_nnan=True,
            nc=nc,
        )
        return tuple(outs)

    devices = jax.devices()[:N]
    mesh = Mesh(np.asarray(devices), ("core",))
    fn = jax.jit(
        shard_map(
            _body,
            mesh=mesh,
            in_specs=(PartitionSpec("core"),),
            out_specs=(PartitionSpec("core"),) * 2,
            check_rep=False,
        )
    )
    _CACHE[key] = fn
    return fn


def _run_device(qx_global: np.ndarray, reps: int = 1):
    """qx_global: (N*2L, C) f32. Returns (ci (N,L,64) u16, pos (N,L,16) u16)."""
    fn = _get_exec(reps)
    ci, pos = fn(qx_global)
    ci = np.asarray(ci).reshape(N, L, NCAND)
    pos = np.asarray(pos).reshape(N, L, K)
    return ci, pos


def _make_qx(coords1: np.ndarray, coords2: np.ndarray) -> np.ndarray:
    # per-core block: rows 0..L-1 = queries (coords2), rows L..2L-1 = refs
    qx = np.empty((N, 2 * L, C), dtype=np.float32)
    qx[:, :L, :] = np.asarray(coords2, dtype=np.float32).transpose(1, 0, 2)
    qx[:, L:, :] = np.asarray(coords1, dtype=np.float32).transpose(1, 0, 2)
    return qx.reshape(N * 2 * L, C)


def _postprocess(ci: np.ndarray, pos: np.ndarray) -> np.ndarray:
    # ci: (N, L, 64) u16 chunk-local indices; pos: (N, L, 16) u16 slots
    slot = pos.astype(np.int64)  # values in [0, 64)
    local = np.take_along_axis(ci.astype(np.int64), slot, axis=2)
    return (slot >> 3) * CHW + local  # global index in [0, 4096)


def kernel(coords1, coords2, k):
    coords1 = np.asarray(coords1)
    coords2 = np.asarray(coords2)
    assert int(k) == K, f"kernel hardcoded for k={K}, got {k}"
    assert coords1.shape == (L, N, C) and coords2.shape == (L, N, C)

    qx = _make_qx(coords1, coords2)
    ci, pos = _run_device(qx)
    local = _postprocess(ci, pos)  # (N, L, K)
    # global_idx = local + n*L1 ; clusters = global_idx mod L2 == local (L1==L2)
    clusters = np.transpose(local, (2, 1, 0)).astype(np.int32).reshape(-1)
    batch_idx = np.broadcast_to(
        np.arange(N, dtype=np.int32), (K, L, N)
    ).reshape(-1)
    return clusters, batch_idx
